# revision 9
# baseline (speedup 1.0000x reference)
"""2-layer GAT (GATConv x2, PyG-style with self-loops) on 8 Trainium2 NeuronCores.

Strategy (graph/data parallel, per sharding hint):
- Nodes are sharded across 8 cores (12500 each, padded to 12544). Each core
  computes the layer projections for its node shard, the per-node attention
  terms (al_src, al_dst) folded into the projection weights on the host, and
  writes a per-node "table" row [al_src | feat] to DRAM.
- Tables are AllGathered so every core holds all source-node rows (halo).
- Incident edges are partitioned by destination. Per 128-destination tile,
  edges sit in slot columns (dest on partitions); each slot column is one
  [128,1] indirect-DMA row gather from the table. Attention softmax and the
  weighted aggregation run as wide strided DVE/ACT ops over the gathered
  block; max-subtraction is skipped (logits are O(1), exp is stable, and
  exp(e-m)/sum == exp(e)/sum analytically).
- Weight matrices are tiny and replicated; a_src/a_dst are folded into the
  projection on the host: W_aug = [W @ Asrc | W | W @ Adst].

Host side does only index/graph preprocessing (shard, degree-sort, slot
packing) plus weight folding; all FLOPs on the N-dimension run on device.
"""

import numpy as np

# Problem constants (hardcoded per spec)
N = 100000
E = 1600000
F_IN = 512
HID = 8
HEADS = 8
F_HID = HID * HEADS  # 64
NUM_CLASSES = 40
NEG_SLOPE = 0.2
CORES = 8
P = 128
BIG_NEG = -1.0e30

_PROGRAM_CACHE = {}


def _split_shards(n, cores):
    base = n // cores
    rem = n % cores
    sizes = [base + (1 if r < rem else 0) for r in range(cores)]
    offs = np.concatenate([[0], np.cumsum(sizes)])
    return sizes, offs


def _preprocess_graph(edge_index, n=N, cores=CORES):
    """Shard dst nodes, degree-sort within shard, pack edge slots.

    Returns dict with per-core index arrays and the shared tile structure.
    """
    src = edge_index[0].astype(np.int64)
    dst = edge_index[1].astype(np.int64)
    # self loops appended, like the reference
    loops = np.arange(n, dtype=np.int64)
    src = np.concatenate([src, loops])
    dst = np.concatenate([dst, loops])

    sizes, offs = _split_shards(n, cores)
    # padded shard size; always keep >=1 pad row (the p=0 dummy target)
    shard_pad = int(np.ceil((max(sizes) + 1) / P) * P)
    tiles = shard_pad // P

    deg = np.bincount(dst, minlength=n)

    # destination shard of each node: contiguous ranges
    node_core = np.searchsorted(offs[1:], np.arange(n), side="right")

    # per-core permutation: own nodes ordered by degree desc (stable)
    perms = []  # perms[r][i] = original node id at padded-local position i
    node_pos = np.zeros(n, dtype=np.int64)  # original node -> local position
    for r in range(cores):
        own = np.arange(offs[r], offs[r + 1])
        order = own[np.argsort(-deg[own], kind="stable")]
        perms.append(order)
        node_pos[order] = np.arange(len(order))

    # table row of node v = core(v)*shard_pad + local_pos(v)
    table_row = node_core * shard_pad + node_pos

    # uniform per-tile slot counts D[t] across cores (max over cores)
    D = np.zeros(tiles, dtype=np.int64)
    for r in range(cores):
        dgs = deg[perms[r]]
        dgs = np.concatenate([dgs, np.zeros(shard_pad - len(dgs), np.int64)])
        dmax = dgs.reshape(tiles, P).max(axis=1)
        D = np.maximum(D, dmax)
    D = np.maximum(D, 1)

    DUMMY = 12500 if shard_pad >= 12544 else (shard_pad - P)  # any padded-row id
    # dummy row must live in the pad region of core 0's shard
    pad_start = int(np.ceil(sizes[0] / 1) )  # real rows count of core 0
    DUMMY = pad_start  # first pad row of core 0's shard

    S = int(D.sum())
    idx_arrays = []
    for r in range(cores):
        idx = np.full((P, S), DUMMY, dtype=np.uint32)
        # edges to this core
        m = (dst >= offs[r]) & (dst < offs[r + 1])
        e_src = src[m]
        e_dst = dst[m]
        lpos = node_pos[e_dst]  # local position of dst in this core
        # sort edges by local dst position for grouped fill
        o = np.argsort(lpos, kind="stable")
        e_src = e_src[o]
        lpos = lpos[o]
        rows = table_row[e_src].astype(np.uint32)
        # slot index within each dst's edge list
        # since sorted by lpos, slot = running index within group
        grp_start = np.searchsorted(lpos, np.arange(shard_pad), side="left")
        grp_end = np.searchsorted(lpos, np.arange(shard_pad), side="right")
        slot = np.arange(len(lpos)) - grp_start[lpos]
        t_of = lpos // P
        part = lpos % P
        off_t = np.concatenate([[0], np.cumsum(D)])[:-1]
        col = off_t[t_of] + slot
        idx[part, col] = rows
        # padded dst rows (no real node): give slot 0 a real row (row 0)
        n_real = sizes[r]
        if shard_pad > n_real:
            pl = np.arange(n_real, shard_pad)
            idx[pl % P, off_t[pl // P]] = 0
        idx_arrays.append(idx)

    return {
        "idx": idx_arrays,  # per-core [P, S] uint32
        "perms": perms,
        "sizes": sizes,
        "offs": offs,
        "shard_pad": shard_pad,
        "tiles": tiles,
        "D": D,
        "S": S,
        "pad_rows_start": [sizes[r] for r in range(cores)],
    }


def _build_program(meta):
    """Build the SPMD Bass program (identical across cores)."""
    from concourse import mybir, bacc
    import concourse.tile as tile
    from concourse.bass import IndirectOffsetOnAxis
    from concourse.masks import make_identity

    dt = mybir.dt
    SH = meta["shard_pad"]
    TILES = meta["tiles"]
    D = meta["D"]
    S = meta["S"]
    NROWS = SH * CORES
    W1C = F_IN // P  # k-chunks for layer-1 matmul
    T1W = HEADS + F_HID  # 72: [al_s(8) | h(64)]
    T2W = 1 + NUM_CLASSES  # 41: [al_s2(1) | g(40)]
    off_t = np.concatenate([[0], np.cumsum(D)])[:-1]

    nc = bacc.Bacc("TRN2", target_bir_lowering=False, debug=False,
                   num_devices=CORES)
    xT = nc.dram_tensor("xT", [F_IN, SH], dt.float32, kind="ExternalInput")
    w1aug = nc.dram_tensor("w1aug", [F_IN, 80], dt.float32, kind="ExternalInput")
    w2aug = nc.dram_tensor("w2aug", [F_HID, 42], dt.float32, kind="ExternalInput")
    b1rep = nc.dram_tensor("b1rep", [P, F_HID], dt.float32, kind="ExternalInput")
    b2rep = nc.dram_tensor("b2rep", [P, NUM_CLASSES], dt.float32, kind="ExternalInput")
    idx_in = nc.dram_tensor("idx", [P, S], dt.uint32, kind="ExternalInput")
    out = nc.dram_tensor("out", [SH, NUM_CLASSES], dt.float32, kind="ExternalOutput")

    AF = mybir.ActivationFunctionType
    OP = mybir.AluOpType
    AX = mybir.AxisListType

    with tile.TileContext(nc) as tc:
        with (
            tc.tile_pool(name="const", bufs=1) as cpool,
            tc.tile_pool(name="resident", bufs=1) as rpool,
            tc.tile_pool(name="work", bufs=3) as wpool,
            tc.tile_pool(name="gbuf", bufs=2) as gpool,
            tc.tile_pool(name="psum", bufs=2, space="PSUM") as ppool,
            tc.tile_pool(name="dram", bufs=1, space="DRAM") as dpool,
        ):
            # ---- constants / residents ----
            w1_t = cpool.tile([P, W1C * 80], dt.float32)
            for c in range(W1C):
                nc.sync.dma_start(out=w1_t[:, c * 80:(c + 1) * 80],
                                  in_=w1aug[c * P:(c + 1) * P, :])
            w2_t = cpool.tile([F_HID, 42], dt.float32)
            nc.sync.dma_start(out=w2_t[:], in_=w2aug[:, :])
            b1_t = cpool.tile([P, F_HID], dt.float32)
            nc.sync.dma_start(out=b1_t[:], in_=b1rep[:, :])
            b2_t = cpool.tile([P, NUM_CLASSES], dt.float32)
            nc.sync.dma_start(out=b2_t[:], in_=b2rep[:, :])
            ident = cpool.tile([P, P], dt.float32)
            make_identity(nc, ident[:])
            idx_t = rpool.tile([P, S], dt.uint32)
            nc.sync.dma_start(out=idx_t[:], in_=idx_in[:, :])
            ald1 = rpool.tile([P, TILES * HEADS], dt.float32)
            ald2 = rpool.tile([P, TILES], dt.float32)

            t1_shard = dpool.tile([SH, T1W], dt.float32)
            t1_full = dpool.tile([NROWS, T1W], dt.float32)
            t2_shard = dpool.tile([SH, T2W], dt.float32)
            t2_full = dpool.tile([NROWS, T2W], dt.float32)

            # ---- phase A: h1 = x @ W1aug per 128-node tile ----
            for t in range(TILES):
                ps = ppool.tile([P, 80], dt.float32, tag="psA")
                for c in range(W1C):
                    lhs = wpool.tile([P, P], dt.float32, tag="xT")
                    nc.sync.dma_start(
                        out=lhs[:],
                        in_=xT[c * P:(c + 1) * P, t * P:(t + 1) * P])
                    nc.tensor.matmul(
                        out=ps[:], lhsT=lhs[:],
                        rhs=w1_t[:, c * 80:(c + 1) * 80],
                        start=(c == 0), stop=(c == W1C - 1))
                row = wpool.tile([P, T1W], dt.float32, tag="t1row")
                nc.scalar.copy(row[:], ps[:, 0:T1W])
                nc.sync.dma_start(out=t1_shard[t * P:(t + 1) * P, :], in_=row[:])
                nc.vector.tensor_copy(ald1[:, t * HEADS:(t + 1) * HEADS],
                                      ps[:, T1W:80])
            # dummy pad rows: al parts -> BIG_NEG so padded slots get p=0
            npad = SH - meta["pad_rows_start"][0]
            if npad > 0:
                dummy = wpool.tile([P, T1W], dt.float32, tag="dummy")
                nc.vector.memset(dummy[:], BIG_NEG)
                nc.sync.dma_start(
                    out=t1_shard[SH - npad:SH, :], in_=dummy[:npad, :])

            nc.gpsimd.collective_compute(
                "AllGather", OP.bypass,
                replica_groups=[list(range(CORES))],
                ins=[t1_shard[:].opt()], outs=[t1_full[:].opt()])

            # ---- phase C1: layer-1 edge aggregation + layer-2 projection ----
            for t in range(TILES):
                Dt = int(D[t])
                o0 = int(off_t[t])
                G = gpool.tile([P, Dt * T1W], dt.float32, tag="G1")
                for j in range(Dt):
                    nc.gpsimd.indirect_dma_start(
                        out=G[:, j * T1W:(j + 1) * T1W],
                        out_offset=None,
                        in_=t1_full[:, :],
                        in_offset=IndirectOffsetOnAxis(
                            ap=idx_t[:, o0 + j:o0 + j + 1], axis=0))
                # attention logits: al_s[src] + al_d[dst]  [P, Dt, HEADS]
                plog = wpool.tile([P, Dt * HEADS], dt.float32, tag="plog")
                G_al = G[:].rearrange("p (d w) -> p d w", w=T1W)[:, :, 0:HEADS]
                ald_b = ald1[:, t * HEADS:(t + 1) * HEADS].unsqueeze(1) \
                    .broadcast_to([P, Dt, HEADS])
                nc.vector.tensor_tensor(
                    out=plog[:].rearrange("p (d w) -> p d w", w=HEADS),
                    in0=G_al, in1=ald_b, op=OP.add)
                nc.vector.scalar_tensor_tensor(
                    out=plog[:], in0=plog[:], scalar=NEG_SLOPE, in1=plog[:],
                    op0=OP.mult, op1=OP.max)
                nc.scalar.activation(plog[:], plog[:], AF.Exp)
                den = wpool.tile([P, HEADS], dt.float32, tag="den")
                nc.vector.tensor_reduce(
                    out=den[:],
                    in_=plog[:].rearrange("p (d w) -> p w d", w=HEADS),
                    axis=AX.X, op=OP.add)
                rec = wpool.tile([P, HEADS], dt.float32, tag="rec")
                nc.vector.reciprocal(rec[:], den[:])
                # weighted features: Gp[p, d, h, f] = G_h * p
                Gp = wpool.tile([P, Dt * F_HID], dt.float32, tag="Gp")
                G_h = G[:].rearrange("p (d w) -> p d w", w=T1W)[:, :, HEADS:T1W] \
                    .rearrange("p d (h f) -> p d h f", f=HID)
                p_b = plog[:].rearrange("p (d h) -> p d h", h=HEADS) \
                    .unsqueeze(3).broadcast_to([P, Dt, HEADS, HID])
                nc.vector.tensor_tensor(
                    out=Gp[:].rearrange("p (d h f) -> p d h f", h=HEADS, f=HID),
                    in0=G_h, in1=p_b, op=OP.mult)
                acc = wpool.tile([P, F_HID], dt.float32, tag="acc")
                nc.vector.tensor_reduce(
                    out=acc[:],
                    in_=Gp[:].rearrange("p (d w) -> p w d", w=F_HID),
                    axis=AX.X, op=OP.add)
                # normalize + bias + elu
                h2 = wpool.tile([P, F_HID], dt.float32, tag="h2")
                rec_b = rec[:].unsqueeze(2).broadcast_to([P, HEADS, HID])
                nc.vector.tensor_tensor(
                    out=h2[:].rearrange("p (h f) -> p h f", f=HID),
                    in0=acc[:].rearrange("p (h f) -> p h f", f=HID),
                    in1=rec_b, op=OP.mult)
                nc.vector.tensor_tensor(out=h2[:], in0=h2[:], in1=b1_t[:], op=OP.add)
                mn = wpool.tile([P, F_HID], dt.float32, tag="mn")
                nc.vector.tensor_scalar_min(mn[:], h2[:], 0.0)
                nc.scalar.activation(mn[:], mn[:], AF.Exp)
                nc.vector.scalar_tensor_tensor(
                    out=h2[:], in0=h2[:], scalar=0.0, in1=mn[:],
                    op0=OP.max, op1=OP.add)
                nc.vector.tensor_scalar_add(h2[:], h2[:], -1.0)
                # layer-2 projection: g = h2 @ W2aug  (transpose h2 via PE)
                pst = ppool.tile([F_HID, P], dt.float32, tag="psT")
                nc.tensor.transpose(out=pst[:], in_=h2[:], identity=ident[:])
                h2T = wpool.tile([F_HID, P], dt.float32, tag="h2T")
                nc.scalar.copy(h2T[:], pst[:])
                ps2 = ppool.tile([P, 42], dt.float32, tag="ps2")
                nc.tensor.matmul(out=ps2[:], lhsT=h2T[:], rhs=w2_t[:],
                                 start=True, stop=True)
                row2 = wpool.tile([P, T2W], dt.float32, tag="t2row")
                nc.scalar.copy(row2[:], ps2[:, 0:T2W])
                nc.sync.dma_start(out=t2_shard[t * P:(t + 1) * P, :], in_=row2[:])
                nc.vector.tensor_copy(ald2[:, t:t + 1], ps2[:, T2W:42])
            if npad > 0:
                dummy2 = wpool.tile([P, T2W], dt.float32, tag="dummy2")
                nc.vector.memset(dummy2[:], BIG_NEG)
                nc.sync.dma_start(
                    out=t2_shard[SH - npad:SH, :], in_=dummy2[:npad, :])

            nc.gpsimd.collective_compute(
                "AllGather", OP.bypass,
                replica_groups=[list(range(CORES))],
                ins=[t2_shard[:].opt()], outs=[t2_full[:].opt()])

            # ---- phase C2: layer-2 edge aggregation + log_softmax ----
            for t in range(TILES):
                Dt = int(D[t])
                o0 = int(off_t[t])
                G2 = gpool.tile([P, Dt * T2W], dt.float32, tag="G2")
                for j in range(Dt):
                    nc.gpsimd.indirect_dma_start(
                        out=G2[:, j * T2W:(j + 1) * T2W],
                        out_offset=None,
                        in_=t2_full[:, :],
                        in_offset=IndirectOffsetOnAxis(
                            ap=idx_t[:, o0 + j:o0 + j + 1], axis=0))
                p2 = wpool.tile([P, Dt], dt.float32, tag="p2")
                nc.vector.tensor_scalar(
                    out=p2[:],
                    in0=G2[:].rearrange("p (d w) -> p d w", w=T2W)[:, :, 0:1].squeeze(2),
                    scalar1=ald2[:, t:t + 1], scalar2=None, op0=OP.add)
                nc.vector.scalar_tensor_tensor(
                    out=p2[:], in0=p2[:], scalar=NEG_SLOPE, in1=p2[:],
                    op0=OP.mult, op1=OP.max)
                den2 = wpool.tile([P, 1], dt.float32, tag="den2")
                nc.scalar.activation(p2[:], p2[:], AF.Exp, accum_out=den2[:])
                rec2 = wpool.tile([P, 1], dt.float32, tag="rec2")
                nc.vector.reciprocal(rec2[:], den2[:])
                G2p = wpool.tile([P, Dt * NUM_CLASSES], dt.float32, tag="G2p")
                G2_h = G2[:].rearrange("p (d w) -> p d w", w=T2W)[:, :, 1:T2W]
                p2_b = p2[:].unsqueeze(2).broadcast_to([P, Dt, NUM_CLASSES])
                nc.vector.tensor_tensor(
                    out=G2p[:].rearrange("p (d w) -> p d w", w=NUM_CLASSES),
                    in0=G2_h, in1=p2_b, op=OP.mult)
                o2 = wpool.tile([P, NUM_CLASSES], dt.float32, tag="o2")
                nc.vector.tensor_reduce(
                    out=o2[:],
                    in_=G2p[:].rearrange("p (d w) -> p w d", w=NUM_CLASSES),
                    axis=AX.X, op=OP.add)
                nc.vector.tensor_scalar(out=o2[:], in0=o2[:], scalar1=rec2[:, 0:1],
                                        scalar2=None, op0=OP.mult)
                nc.vector.tensor_tensor(out=o2[:], in0=o2[:], in1=b2_t[:], op=OP.add)
                # log_softmax over classes
                mx = wpool.tile([P, 1], dt.float32, tag="mx")
                nc.vector.tensor_reduce(out=mx[:], in_=o2[:], axis=AX.X, op=OP.max)
                nc.vector.tensor_scalar(out=o2[:], in0=o2[:], scalar1=mx[:, 0:1],
                                        scalar2=None, op0=OP.subtract)
                ex = wpool.tile([P, NUM_CLASSES], dt.float32, tag="ex")
                sm = wpool.tile([P, 1], dt.float32, tag="sm")
                nc.scalar.activation(ex[:], o2[:], AF.Exp, accum_out=sm[:])
                lg = wpool.tile([P, 1], dt.float32, tag="lg")
                nc.scalar.activation(lg[:], sm[:], AF.Ln)
                nc.vector.tensor_scalar(out=o2[:], in0=o2[:], scalar1=lg[:, 0:1],
                                        scalar2=None, op0=OP.subtract)
                nc.sync.dma_start(out=out[t * P:(t + 1) * P, :], in_=o2[:])
    nc.compile()
    return nc


def _make_runner(nc, n_cores=CORES):
    """Hold a jitted PJRT executable for repeated invocation."""
    import jax
    from jax.sharding import Mesh, PartitionSpec
    from jax.experimental.shard_map import shard_map
    from concourse import mybir
    from concourse.bass2jax import (_bass_exec_p, install_neuronx_cc_hook,
                                    partition_id_tensor)
    install_neuronx_cc_hook()
    partition_name = nc.partition_id_tensor.name if nc.partition_id_tensor else None
    in_names, out_names, out_avals, zero_outs = [], [], [], []
    for alloc in nc.m.functions[0].allocations:
        if not isinstance(alloc, mybir.MemoryLocationSet):
            continue
        name = alloc.memorylocations[0].name
        if alloc.kind == "ExternalInput":
            if name != partition_name:
                in_names.append(name)
        elif alloc.kind == "ExternalOutput":
            shape = tuple(alloc.tensor_shape)
            dtype = mybir.dt.np(alloc.dtype)
            out_names.append(name)
            out_avals.append(jax.core.ShapedArray(shape, dtype))
            zero_outs.append(np.zeros(shape, dtype))
    n_params = len(in_names)
    all_in = list(in_names) + list(out_names) + ([partition_name] if partition_name else [])

    def _body(*args):
        operands = list(args)
        if partition_name is not None:
            operands.append(partition_id_tensor())
        outs = _bass_exec_p.bind(
            *operands, out_avals=tuple(out_avals), in_names=tuple(all_in),
            out_names=tuple(out_names), lowering_input_output_aliases=(),
            sim_require_finite=True, sim_require_nnan=True, nc=nc)
        return tuple(outs)

    devices = jax.devices()[:n_cores]
    mesh = Mesh(np.asarray(devices), ("core",))
    nio = n_params + len(out_names)
    sharded = jax.jit(
        shard_map(_body, mesh=mesh, in_specs=(PartitionSpec("core"),) * nio,
                  out_specs=(PartitionSpec("core"),) * len(out_names),
                  check_rep=False),
        keep_unused=True)

    def run(in_maps, time_reps=0):
        import time as _t
        concat_in = [np.concatenate([np.asarray(in_maps[c][nm])
                                     for c in range(n_cores)], axis=0)
                     for nm in in_names]
        concat_zero = [np.zeros((n_cores * z.shape[0], *z.shape[1:]), z.dtype)
                       for z in zero_outs]
        dev_in = [jax.device_put(a) for a in concat_in]
        dev_zero = [jax.device_put(a) for a in concat_zero]
        outs = sharded(*dev_in, *dev_zero)
        jax.block_until_ready(outs)
        tmin = None
        if time_reps:
            ts = []
            for _ in range(time_reps):
                t0 = _t.perf_counter()
                outs = sharded(*dev_in, *dev_zero)
                jax.block_until_ready(outs)
                ts.append(_t.perf_counter() - t0)
            tmin = min(ts)
        results = [{nm: np.asarray(outs[i]).reshape(n_cores, *out_avals[i].shape)[c]
                    for i, nm in enumerate(out_names)} for c in range(n_cores)]
        return results, tmin

    return run


def kernel(x, edge_index, W1, a_src1, a_dst1, b1, W2, a_src2, a_dst2, b2,
           _time_reps=0):
    x = np.asarray(x, dtype=np.float32)
    edge_index = np.asarray(edge_index)
    W1 = np.asarray(W1, dtype=np.float32)
    W2 = np.asarray(W2, dtype=np.float32)
    a_src1 = np.asarray(a_src1, dtype=np.float32)
    a_dst1 = np.asarray(a_dst1, dtype=np.float32)
    a_src2 = np.asarray(a_src2, dtype=np.float32)
    a_dst2 = np.asarray(a_dst2, dtype=np.float32)
    b1 = np.asarray(b1, dtype=np.float32)
    b2 = np.asarray(b2, dtype=np.float32)

    meta = _preprocess_graph(edge_index)
    SH = meta["shard_pad"]

    # fold attention vectors into the projections (host, tiny)
    As = np.zeros((F_HID, HEADS), dtype=np.float32)
    Ad = np.zeros((F_HID, HEADS), dtype=np.float32)
    for h in range(HEADS):
        As[h * HID:(h + 1) * HID, h] = a_src1[h]
        Ad[h * HID:(h + 1) * HID, h] = a_dst1[h]
    w1aug = np.concatenate([W1 @ As, W1, W1 @ Ad], axis=1)  # [512, 80]
    w2aug = np.concatenate([W2 @ a_src2.T, W2, W2 @ a_dst2.T], axis=1)  # [64, 42]
    b1rep = np.broadcast_to(b1[None, :], (P, F_HID)).copy()
    b2rep = np.broadcast_to(b2[None, :], (P, NUM_CLASSES)).copy()

    key = (tuple(meta["D"].tolist()), SH)
    if key not in _PROGRAM_CACHE:
        nc = _build_program(meta)
        _PROGRAM_CACHE[key] = _make_runner(nc)
    run = _PROGRAM_CACHE[key]

    in_maps = []
    for r in range(CORES):
        perm = meta["perms"][r]
        xs = np.zeros((SH, F_IN), dtype=np.float32)
        xs[:len(perm)] = x[perm]
        in_maps.append({
            "xT": np.ascontiguousarray(xs.T),
            "w1aug": w1aug, "w2aug": w2aug,
            "b1rep": b1rep, "b2rep": b2rep,
            "idx": meta["idx"][r],
        })

    results, tmin = run(in_maps, time_reps=_time_reps)
    out = np.zeros((N, NUM_CLASSES), dtype=np.float32)
    for r in range(CORES):
        perm = meta["perms"][r]
        out[perm] = results[r]["out"][:len(perm)]
    if _time_reps:
        kernel._last_time_s = tmin
    return out


# revision 14
# speedup vs baseline: 1.0468x; 1.0468x over previous
"""2-layer GAT (GATConv x2, PyG-style with self-loops) on 8 Trainium2 NeuronCores.

Strategy (graph/data parallel, per sharding hint):
- Nodes are sharded across 8 cores (12500 each, padded to 12544). Each core
  computes the layer projections for its node shard, the per-node attention
  terms (al_src, al_dst) folded into the projection weights on the host, and
  writes a per-node "table" row [al_src | feat] to DRAM.
- Tables are AllGathered so every core holds all source-node rows (halo).
- Incident edges are partitioned by destination. Per 128-destination tile,
  edges sit in slot columns (dest on partitions); each slot column is one
  [128,1] indirect-DMA row gather from the table. Attention softmax and the
  weighted aggregation run as wide strided DVE/ACT ops over the gathered
  block; max-subtraction is skipped (logits are O(1), exp is stable, and
  exp(e-m)/sum == exp(e)/sum analytically).
- Weight matrices are tiny and replicated; a_src/a_dst are folded into the
  projection on the host: W_aug = [W @ Asrc | W | W @ Adst].

Host side does only index/graph preprocessing (shard, degree-sort, slot
packing) plus weight folding; all FLOPs on the N-dimension run on device.
"""

import numpy as np

# Problem constants (hardcoded per spec)
N = 100000
E = 1600000
F_IN = 512
HID = 8
HEADS = 8
F_HID = HID * HEADS  # 64
NUM_CLASSES = 40
NEG_SLOPE = 0.2
CORES = 8
P = 128
BIG_NEG = -1.0e30

_PROGRAM_CACHE = {}


def _split_shards(n, cores):
    base = n // cores
    rem = n % cores
    sizes = [base + (1 if r < rem else 0) for r in range(cores)]
    offs = np.concatenate([[0], np.cumsum(sizes)])
    return sizes, offs


def _preprocess_graph(edge_index, n=N, cores=CORES):
    """Shard dst nodes, degree-sort within shard, pack edge slots.

    Returns dict with per-core index arrays and the shared tile structure.
    """
    # The reference appends a self-loop per node; we serve those via a static
    # contiguous DMA per tile (slot D[t]) instead of indirect gathers, so only
    # the original random edges go through the slot structure here.
    src = edge_index[0].astype(np.int64)
    dst = edge_index[1].astype(np.int64)

    sizes, offs = _split_shards(n, cores)
    # padded shard size; always keep >=1 pad row (the p=0 dummy target)
    shard_pad = int(np.ceil((max(sizes) + 1) / P) * P)
    tiles = shard_pad // P

    deg = np.bincount(dst, minlength=n)

    # destination shard of each node: contiguous ranges
    node_core = np.searchsorted(offs[1:], np.arange(n), side="right")

    # per-core permutation: own nodes ordered by degree desc (stable)
    perms = []  # perms[r][i] = original node id at padded-local position i
    node_pos = np.zeros(n, dtype=np.int64)  # original node -> local position
    for r in range(cores):
        own = np.arange(offs[r], offs[r + 1])
        order = own[np.argsort(-deg[own], kind="stable")]
        perms.append(order)
        node_pos[order] = np.arange(len(order))

    # table row of node v = core(v)*shard_pad + local_pos(v)
    table_row = node_core * shard_pad + node_pos

    # uniform per-tile slot counts D[t] across cores (max over cores)
    D = np.zeros(tiles, dtype=np.int64)
    for r in range(cores):
        dgs = deg[perms[r]]
        dgs = np.concatenate([dgs, np.zeros(shard_pad - len(dgs), np.int64)])
        dmax = dgs.reshape(tiles, P).max(axis=1)
        D = np.maximum(D, dmax)
    D = np.maximum(D, 1)

    DUMMY = 12500 if shard_pad >= 12544 else (shard_pad - P)  # any padded-row id
    # dummy row must live in the pad region of core 0's shard
    pad_start = int(np.ceil(sizes[0] / 1) )  # real rows count of core 0
    DUMMY = pad_start  # first pad row of core 0's shard

    S = int(D.sum())
    idx_arrays = []
    for r in range(cores):
        idx = np.full((P, S), DUMMY, dtype=np.uint32)
        # edges to this core
        m = (dst >= offs[r]) & (dst < offs[r + 1])
        e_src = src[m]
        e_dst = dst[m]
        lpos = node_pos[e_dst]  # local position of dst in this core
        # sort edges by local dst position for grouped fill
        o = np.argsort(lpos, kind="stable")
        e_src = e_src[o]
        lpos = lpos[o]
        rows = table_row[e_src].astype(np.uint32)
        # slot index within each dst's edge list
        # since sorted by lpos, slot = running index within group
        grp_start = np.searchsorted(lpos, np.arange(shard_pad), side="left")
        grp_end = np.searchsorted(lpos, np.arange(shard_pad), side="right")
        slot = np.arange(len(lpos)) - grp_start[lpos]
        t_of = lpos // P
        part = lpos % P
        off_t = np.concatenate([[0], np.cumsum(D)])[:-1]
        col = off_t[t_of] + slot
        idx[part, col] = rows
        # padded dst rows (no real node): give slot 0 a real row (row 0)
        n_real = sizes[r]
        if shard_pad > n_real:
            pl = np.arange(n_real, shard_pad)
            idx[pl % P, off_t[pl // P]] = 0
        idx_arrays.append(idx)

    return {
        "idx": idx_arrays,  # per-core [P, S] uint32
        "perms": perms,
        "sizes": sizes,
        "offs": offs,
        "shard_pad": shard_pad,
        "tiles": tiles,
        "D": D,
        "S": S,
        "pad_rows_start": [sizes[r] for r in range(cores)],
    }


def _build_program(meta):
    """Build the SPMD Bass program (identical across cores)."""
    from concourse import mybir, bacc
    import concourse.tile as tile
    from concourse.bass import IndirectOffsetOnAxis
    from concourse.masks import make_identity

    dt = mybir.dt
    SH = meta["shard_pad"]
    TILES = meta["tiles"]
    D = meta["D"]
    S = meta["S"]
    NROWS = SH * CORES
    W1C = F_IN // P  # k-chunks for layer-1 matmul
    T1W = HEADS + F_HID  # 72: [al_s(8) | h(64)]
    T2W = 1 + NUM_CLASSES  # 41: [al_s2(1) | g(40)]
    off_t = np.concatenate([[0], np.cumsum(D)])[:-1]

    nc = bacc.Bacc("TRN2", target_bir_lowering=False, debug=False,
                   num_devices=CORES)
    xT = nc.dram_tensor("xT", [F_IN, SH], dt.float32, kind="ExternalInput")
    w1aug = nc.dram_tensor("w1aug", [F_IN, 80], dt.float32, kind="ExternalInput")
    w2aug = nc.dram_tensor("w2aug", [F_HID, 42], dt.float32, kind="ExternalInput")
    b1rep = nc.dram_tensor("b1rep", [P, F_HID], dt.float32, kind="ExternalInput")
    b2rep = nc.dram_tensor("b2rep", [P, NUM_CLASSES], dt.float32, kind="ExternalInput")
    idx_in = nc.dram_tensor("idx", [P, S], dt.uint32, kind="ExternalInput")
    out = nc.dram_tensor("out", [SH, NUM_CLASSES], dt.float32, kind="ExternalOutput")

    AF = mybir.ActivationFunctionType
    OP = mybir.AluOpType
    AX = mybir.AxisListType

    with tile.TileContext(nc) as tc:
        with (
            tc.tile_pool(name="const", bufs=1) as cpool,
            tc.tile_pool(name="resident", bufs=1) as rpool,
            tc.tile_pool(name="work", bufs=3) as wpool,
            tc.tile_pool(name="gbuf", bufs=2) as gpool,
            tc.tile_pool(name="psum", bufs=2, space="PSUM") as ppool,
            tc.tile_pool(name="dram", bufs=1, space="DRAM") as dpool,
        ):
            # ---- constants / residents ----
            w1_t = cpool.tile([P, W1C * 80], dt.float32)
            for c in range(W1C):
                nc.sync.dma_start(out=w1_t[:, c * 80:(c + 1) * 80],
                                  in_=w1aug[c * P:(c + 1) * P, :])
            w2_t = cpool.tile([F_HID, 42], dt.float32)
            nc.sync.dma_start(out=w2_t[:], in_=w2aug[:, :])
            b1_t = cpool.tile([P, F_HID], dt.float32)
            nc.sync.dma_start(out=b1_t[:], in_=b1rep[:, :])
            b2_t = cpool.tile([P, NUM_CLASSES], dt.float32)
            nc.sync.dma_start(out=b2_t[:], in_=b2rep[:, :])
            ident = cpool.tile([P, P], dt.float32)
            make_identity(nc, ident[:])
            idx_t = rpool.tile([P, S], dt.uint32)
            nc.sync.dma_start(out=idx_t[:], in_=idx_in[:, :])
            ald1 = rpool.tile([P, TILES * HEADS], dt.float32)
            ald2 = rpool.tile([P, TILES], dt.float32)

            t1_shard = dpool.tile([SH, T1W], dt.float32)
            t1_full = dpool.tile([NROWS, T1W], dt.float32)
            t2_shard = dpool.tile([SH, T2W], dt.float32)
            t2_full = dpool.tile([NROWS, T2W], dt.float32)

            # ---- phase A: h1 = x @ W1aug per 128-node tile ----
            for t in range(TILES):
                ps = ppool.tile([P, 80], dt.float32, tag="psA")
                for c in range(W1C):
                    lhs = wpool.tile([P, P], dt.float32, tag="xT")
                    nc.sync.dma_start(
                        out=lhs[:],
                        in_=xT[c * P:(c + 1) * P, t * P:(t + 1) * P])
                    nc.tensor.matmul(
                        out=ps[:], lhsT=lhs[:],
                        rhs=w1_t[:, c * 80:(c + 1) * 80],
                        start=(c == 0), stop=(c == W1C - 1))
                row = wpool.tile([P, T1W], dt.float32, tag="t1row")
                nc.scalar.copy(row[:], ps[:, 0:T1W])
                nc.sync.dma_start(out=t1_shard[t * P:(t + 1) * P, :], in_=row[:])
                nc.vector.tensor_copy(ald1[:, t * HEADS:(t + 1) * HEADS],
                                      ps[:, T1W:80])
            # dummy pad rows: al parts -> BIG_NEG so padded slots get p=0
            npad = SH - meta["pad_rows_start"][0]
            if npad > 0:
                dummy = wpool.tile([P, T1W], dt.float32, tag="dummy")
                nc.vector.memset(dummy[:], BIG_NEG)
                nc.sync.dma_start(
                    out=t1_shard[SH - npad:SH, :], in_=dummy[:npad, :])

            nc.gpsimd.collective_compute(
                "AllGather", OP.bypass,
                replica_groups=[list(range(CORES))],
                ins=[t1_shard[:].opt()], outs=[t1_full[:].opt()])

            # ---- phase C1: layer-1 edge aggregation + layer-2 projection ----
            for t in range(TILES):
                Dt = int(D[t])
                SL = Dt + 1  # last slot = self-loop, loaded contiguously
                o0 = int(off_t[t])
                G = gpool.tile([P, SL * T1W], dt.float32, tag="G1")
                for j in range(Dt):
                    nc.gpsimd.indirect_dma_start(
                        out=G[:, j * T1W:(j + 1) * T1W],
                        out_offset=None,
                        in_=t1_full[:, :],
                        in_offset=IndirectOffsetOnAxis(
                            ap=idx_t[:, o0 + j:o0 + j + 1], axis=0))
                # self-loop slot: own shard rows are contiguous -> static DMA
                nc.sync.dma_start(
                    out=G[:, Dt * T1W:SL * T1W],
                    in_=t1_shard[t * P:(t + 1) * P, :])
                # attention logits: al_s[src] + al_d[dst]  [P, SL, HEADS]
                plog = wpool.tile([P, SL * HEADS], dt.float32, tag="plog")
                G_al = G[:].rearrange("p (d w) -> p d w", w=T1W)[:, :, 0:HEADS]
                ald_b = ald1[:, t * HEADS:(t + 1) * HEADS].unsqueeze(1) \
                    .broadcast_to([P, SL, HEADS])
                nc.vector.tensor_tensor(
                    out=plog[:].rearrange("p (d w) -> p d w", w=HEADS),
                    in0=G_al, in1=ald_b, op=OP.add)
                nc.vector.scalar_tensor_tensor(
                    out=plog[:], in0=plog[:], scalar=NEG_SLOPE, in1=plog[:],
                    op0=OP.mult, op1=OP.max)
                nc.scalar.activation(plog[:], plog[:], AF.Exp)
                den = wpool.tile([P, HEADS], dt.float32, tag="den")
                nc.vector.tensor_reduce(
                    out=den[:],
                    in_=plog[:].rearrange("p (d w) -> p w d", w=HEADS),
                    axis=AX.X, op=OP.add)
                rec = wpool.tile([P, HEADS], dt.float32, tag="rec")
                nc.vector.reciprocal(rec[:], den[:])
                # weighted features: Gp[p, d, h, f] = G_h * p
                Gp = wpool.tile([P, SL * F_HID], dt.float32, tag="Gp")
                G_h = G[:].rearrange("p (d w) -> p d w", w=T1W)[:, :, HEADS:T1W] \
                    .rearrange("p d (h f) -> p d h f", f=HID)
                p_b = plog[:].rearrange("p (d h) -> p d h", h=HEADS) \
                    .unsqueeze(3).broadcast_to([P, SL, HEADS, HID])
                nc.vector.tensor_tensor(
                    out=Gp[:].rearrange("p (d h f) -> p d h f", h=HEADS, f=HID),
                    in0=G_h, in1=p_b, op=OP.mult)
                acc = wpool.tile([P, F_HID], dt.float32, tag="acc")
                nc.vector.tensor_reduce(
                    out=acc[:],
                    in_=Gp[:].rearrange("p (d w) -> p w d", w=F_HID),
                    axis=AX.X, op=OP.add)
                # normalize + bias + elu
                h2 = wpool.tile([P, F_HID], dt.float32, tag="h2")
                rec_b = rec[:].unsqueeze(2).broadcast_to([P, HEADS, HID])
                nc.vector.tensor_tensor(
                    out=h2[:].rearrange("p (h f) -> p h f", f=HID),
                    in0=acc[:].rearrange("p (h f) -> p h f", f=HID),
                    in1=rec_b, op=OP.mult)
                nc.vector.tensor_tensor(out=h2[:], in0=h2[:], in1=b1_t[:], op=OP.add)
                mn = wpool.tile([P, F_HID], dt.float32, tag="mn")
                nc.vector.tensor_scalar_min(mn[:], h2[:], 0.0)
                nc.scalar.activation(mn[:], mn[:], AF.Exp)
                nc.vector.scalar_tensor_tensor(
                    out=h2[:], in0=h2[:], scalar=0.0, in1=mn[:],
                    op0=OP.max, op1=OP.add)
                nc.vector.tensor_scalar_add(h2[:], h2[:], -1.0)
                # layer-2 projection: g = h2 @ W2aug  (transpose h2 via PE)
                pst = ppool.tile([F_HID, P], dt.float32, tag="psT")
                nc.tensor.transpose(out=pst[:], in_=h2[:], identity=ident[:])
                h2T = wpool.tile([F_HID, P], dt.float32, tag="h2T")
                nc.scalar.copy(h2T[:], pst[:])
                ps2 = ppool.tile([P, 42], dt.float32, tag="ps2")
                nc.tensor.matmul(out=ps2[:], lhsT=h2T[:], rhs=w2_t[:],
                                 start=True, stop=True)
                row2 = wpool.tile([P, T2W], dt.float32, tag="t2row")
                nc.scalar.copy(row2[:], ps2[:, 0:T2W])
                nc.sync.dma_start(out=t2_shard[t * P:(t + 1) * P, :], in_=row2[:])
                nc.vector.tensor_copy(ald2[:, t:t + 1], ps2[:, T2W:42])
            if npad > 0:
                dummy2 = wpool.tile([P, T2W], dt.float32, tag="dummy2")
                nc.vector.memset(dummy2[:], BIG_NEG)
                nc.sync.dma_start(
                    out=t2_shard[SH - npad:SH, :], in_=dummy2[:npad, :])

            nc.gpsimd.collective_compute(
                "AllGather", OP.bypass,
                replica_groups=[list(range(CORES))],
                ins=[t2_shard[:].opt()], outs=[t2_full[:].opt()])

            # ---- phase C2: layer-2 edge aggregation + log_softmax ----
            for t in range(TILES):
                Dt = int(D[t])
                SL = Dt + 1
                o0 = int(off_t[t])
                G2 = gpool.tile([P, SL * T2W], dt.float32, tag="G2")
                for j in range(Dt):
                    nc.gpsimd.indirect_dma_start(
                        out=G2[:, j * T2W:(j + 1) * T2W],
                        out_offset=None,
                        in_=t2_full[:, :],
                        in_offset=IndirectOffsetOnAxis(
                            ap=idx_t[:, o0 + j:o0 + j + 1], axis=0))
                nc.sync.dma_start(
                    out=G2[:, Dt * T2W:SL * T2W],
                    in_=t2_shard[t * P:(t + 1) * P, :])
                p2 = wpool.tile([P, SL], dt.float32, tag="p2")
                nc.vector.tensor_scalar(
                    out=p2[:],
                    in0=G2[:].rearrange("p (d w) -> p d w", w=T2W)[:, :, 0:1].squeeze(2),
                    scalar1=ald2[:, t:t + 1], scalar2=None, op0=OP.add)
                nc.vector.scalar_tensor_tensor(
                    out=p2[:], in0=p2[:], scalar=NEG_SLOPE, in1=p2[:],
                    op0=OP.mult, op1=OP.max)
                den2 = wpool.tile([P, 1], dt.float32, tag="den2")
                nc.scalar.activation(p2[:], p2[:], AF.Exp, accum_out=den2[:])
                rec2 = wpool.tile([P, 1], dt.float32, tag="rec2")
                nc.vector.reciprocal(rec2[:], den2[:])
                G2p = wpool.tile([P, SL * NUM_CLASSES], dt.float32, tag="G2p")
                G2_h = G2[:].rearrange("p (d w) -> p d w", w=T2W)[:, :, 1:T2W]
                p2_b = p2[:].unsqueeze(2).broadcast_to([P, SL, NUM_CLASSES])
                nc.vector.tensor_tensor(
                    out=G2p[:].rearrange("p (d w) -> p d w", w=NUM_CLASSES),
                    in0=G2_h, in1=p2_b, op=OP.mult)
                o2 = wpool.tile([P, NUM_CLASSES], dt.float32, tag="o2")
                nc.vector.tensor_reduce(
                    out=o2[:],
                    in_=G2p[:].rearrange("p (d w) -> p w d", w=NUM_CLASSES),
                    axis=AX.X, op=OP.add)
                nc.vector.tensor_scalar(out=o2[:], in0=o2[:], scalar1=rec2[:, 0:1],
                                        scalar2=None, op0=OP.mult)
                nc.vector.tensor_tensor(out=o2[:], in0=o2[:], in1=b2_t[:], op=OP.add)
                # log_softmax over classes
                mx = wpool.tile([P, 1], dt.float32, tag="mx")
                nc.vector.tensor_reduce(out=mx[:], in_=o2[:], axis=AX.X, op=OP.max)
                nc.vector.tensor_scalar(out=o2[:], in0=o2[:], scalar1=mx[:, 0:1],
                                        scalar2=None, op0=OP.subtract)
                ex = wpool.tile([P, NUM_CLASSES], dt.float32, tag="ex")
                sm = wpool.tile([P, 1], dt.float32, tag="sm")
                nc.scalar.activation(ex[:], o2[:], AF.Exp, accum_out=sm[:])
                lg = wpool.tile([P, 1], dt.float32, tag="lg")
                nc.scalar.activation(lg[:], sm[:], AF.Ln)
                nc.vector.tensor_scalar(out=o2[:], in0=o2[:], scalar1=lg[:, 0:1],
                                        scalar2=None, op0=OP.subtract)
                nc.sync.dma_start(out=out[t * P:(t + 1) * P, :], in_=o2[:])
    nc.compile()
    return nc


def _make_runner(nc, n_cores=CORES):
    """Hold a jitted PJRT executable for repeated invocation."""
    import jax
    from jax.sharding import Mesh, PartitionSpec
    from jax.experimental.shard_map import shard_map
    from concourse import mybir
    from concourse.bass2jax import (_bass_exec_p, install_neuronx_cc_hook,
                                    partition_id_tensor)
    install_neuronx_cc_hook()
    partition_name = nc.partition_id_tensor.name if nc.partition_id_tensor else None
    in_names, out_names, out_avals, zero_outs = [], [], [], []
    for alloc in nc.m.functions[0].allocations:
        if not isinstance(alloc, mybir.MemoryLocationSet):
            continue
        name = alloc.memorylocations[0].name
        if alloc.kind == "ExternalInput":
            if name != partition_name:
                in_names.append(name)
        elif alloc.kind == "ExternalOutput":
            shape = tuple(alloc.tensor_shape)
            dtype = mybir.dt.np(alloc.dtype)
            out_names.append(name)
            out_avals.append(jax.core.ShapedArray(shape, dtype))
            zero_outs.append(np.zeros(shape, dtype))
    n_params = len(in_names)
    all_in = list(in_names) + list(out_names) + ([partition_name] if partition_name else [])

    def _body(*args):
        operands = list(args)
        if partition_name is not None:
            operands.append(partition_id_tensor())
        outs = _bass_exec_p.bind(
            *operands, out_avals=tuple(out_avals), in_names=tuple(all_in),
            out_names=tuple(out_names), lowering_input_output_aliases=(),
            sim_require_finite=True, sim_require_nnan=True, nc=nc)
        return tuple(outs)

    devices = jax.devices()[:n_cores]
    mesh = Mesh(np.asarray(devices), ("core",))
    nio = n_params + len(out_names)
    sharded = jax.jit(
        shard_map(_body, mesh=mesh, in_specs=(PartitionSpec("core"),) * nio,
                  out_specs=(PartitionSpec("core"),) * len(out_names),
                  check_rep=False),
        keep_unused=True)

    def run(in_maps, time_reps=0):
        import time as _t
        concat_in = [np.concatenate([np.asarray(in_maps[c][nm])
                                     for c in range(n_cores)], axis=0)
                     for nm in in_names]
        concat_zero = [np.zeros((n_cores * z.shape[0], *z.shape[1:]), z.dtype)
                       for z in zero_outs]
        dev_in = [jax.device_put(a) for a in concat_in]
        dev_zero = [jax.device_put(a) for a in concat_zero]
        outs = sharded(*dev_in, *dev_zero)
        jax.block_until_ready(outs)
        tmin = None
        if time_reps:
            ts = []
            for _ in range(time_reps):
                t0 = _t.perf_counter()
                outs = sharded(*dev_in, *dev_zero)
                jax.block_until_ready(outs)
                ts.append(_t.perf_counter() - t0)
            tmin = min(ts)
        results = [{nm: np.asarray(outs[i]).reshape(n_cores, *out_avals[i].shape)[c]
                    for i, nm in enumerate(out_names)} for c in range(n_cores)]
        return results, tmin

    return run


def kernel(x, edge_index, W1, a_src1, a_dst1, b1, W2, a_src2, a_dst2, b2,
           _time_reps=0):
    x = np.asarray(x, dtype=np.float32)
    edge_index = np.asarray(edge_index)
    W1 = np.asarray(W1, dtype=np.float32)
    W2 = np.asarray(W2, dtype=np.float32)
    a_src1 = np.asarray(a_src1, dtype=np.float32)
    a_dst1 = np.asarray(a_dst1, dtype=np.float32)
    a_src2 = np.asarray(a_src2, dtype=np.float32)
    a_dst2 = np.asarray(a_dst2, dtype=np.float32)
    b1 = np.asarray(b1, dtype=np.float32)
    b2 = np.asarray(b2, dtype=np.float32)

    meta = _preprocess_graph(edge_index)
    SH = meta["shard_pad"]

    # fold attention vectors into the projections (host, tiny)
    As = np.zeros((F_HID, HEADS), dtype=np.float32)
    Ad = np.zeros((F_HID, HEADS), dtype=np.float32)
    for h in range(HEADS):
        As[h * HID:(h + 1) * HID, h] = a_src1[h]
        Ad[h * HID:(h + 1) * HID, h] = a_dst1[h]
    w1aug = np.concatenate([W1 @ As, W1, W1 @ Ad], axis=1)  # [512, 80]
    w2aug = np.concatenate([W2 @ a_src2.T, W2, W2 @ a_dst2.T], axis=1)  # [64, 42]
    b1rep = np.broadcast_to(b1[None, :], (P, F_HID)).copy()
    b2rep = np.broadcast_to(b2[None, :], (P, NUM_CLASSES)).copy()

    key = (tuple(meta["D"].tolist()), SH)
    if key not in _PROGRAM_CACHE:
        nc = _build_program(meta)
        _PROGRAM_CACHE[key] = _make_runner(nc)
    run = _PROGRAM_CACHE[key]

    in_maps = []
    for r in range(CORES):
        perm = meta["perms"][r]
        xs = np.zeros((SH, F_IN), dtype=np.float32)
        xs[:len(perm)] = x[perm]
        in_maps.append({
            "xT": np.ascontiguousarray(xs.T),
            "w1aug": w1aug, "w2aug": w2aug,
            "b1rep": b1rep, "b2rep": b2rep,
            "idx": meta["idx"][r],
        })

    results, tmin = run(in_maps, time_reps=_time_reps)
    out = np.zeros((N, NUM_CLASSES), dtype=np.float32)
    for r in range(CORES):
        perm = meta["perms"][r]
        out[perm] = results[r]["out"][:len(perm)]
    if _time_reps:
        kernel._last_time_s = tmin
    return out
